# revision 33
# baseline (speedup 1.0000x reference)
"""Trainium2 Bass kernel for nn_DA3CrossFrameRKDDistanceLoss (v3).

Math (reference semantics): ref rows (teacher/student frame 0, ref_perm
subsample), extra = teacher frames [1,3,5,7] concat -> [4096, D].  Cosine
top-4 neighbours of each ref row inside extra; KL(softmax(diff_t) ||
softmax(diff_s)) per row with diff pairs (d1: ref-shared, d2: ref-simhigh,
d3: shared-simhigh), smooth-L1 (beta=0.5) of each KL, averaged per branch
and summed.  kl = S/Sa - ln Sa + ln Sb with a = diff_t, b = diff_s,
S = sum(exp(a)*(a-b)), Sa = sum(exp(a)), Sb = sum(exp(b)).  d2 in the
reference loop is identical across the 3 shared frames -> computed once.

Sharding: 8 cores = (batch b in 0..3) x (half h of the 256 ref rows); no
collectives — the host sums the per-core smooth-L1 outputs.

Device pipeline per core:
  1. fp8(e4m3) similarity matmul (refn*16 @ extn*16.T), streamed in 8
     PSUM-bank chunks; per chunk: top8 values (max8) + in-chunk indices
     (max_index) directly from PSUM, folded into an order-preserving
     encoding enc = (round(4*v)+1536)*4096 + global_idx (fp32-exact
     integers; ties impossible since indices differ).  Numerically
     validated: fp8+quantized selection moves the loss by ~1e-4 rel.
  2. Global top4 = first 4 of max8(enc); index decoded arithmetically
     (no 4096-wide max_index on the critical path).
  3. Four indirect row gathers of the winners from a bf16 copy of extra.
  4. KL phase: a-sides direct (sub -> ACT exp with accum_out giving Sa
     for free); b-sides shared-exponential (exp(y - sh) = e_y * e_nsh,
     product on DVE/Pool, sum via tensor_scalar accum_out at 4x);
     S = sum(ea * cv) likewise.  e_y / e_nss / cv / c2-c3 are computed
     once, batched, during the gather window together with the d1 units.
  5. smooth-L1 tail -> hub [128, 19] -> host reduction.

Container quirks honoured here: walrus allows ONE sync-wait per compute
instruction (_split_waits rewrites extras onto same-engine Drain carriers);
tensor_tensor_reduce / custom-DVE ops hit 'ISA wrong length'; ALU mod is
rejected; max8/max_index only safe on fp32; multi-offset indirect DMA
fails — but tensor_scalar/scalar_tensor_tensor with accum_out DO work.
"""

import os
import sys

import numpy as np

for _p in ("/opt/trn_rl_repo", "/root/.axon_site/_ro/trn_rl_repo"):
    # later inserts go to the front: prefer the axon-site copy when present
    if os.path.isdir(_p) and _p not in sys.path:
        sys.path.insert(0, _p)

import concourse.bass as bass
import concourse.tile as tile
from concourse import mybir
from concourse.bass_utils import run_bass_kernel_spmd

F32 = mybir.dt.float32
BF16 = mybir.dt.bfloat16
FP8 = mybir.dt.float8e4
I32 = mybir.dt.int32
U16 = mybir.dt.uint16

B = 4
P = 1024
D = 1024
NUM_REF = 256
TOPK = 4
NREF_CORE = 128
NEXTRA = 4 * P
EXTRA_FRAMES = (1, 3, 5, 7)
SHARED_T = (2, 4, 6)
SHARED_S = (1, 2, 3)
NFRAMES = 3
N_UNITS = 19              # 3 d1 + 16 (j,f): col = 3 + 4j + f
N_CHUNK = 8
CHUNK = NEXTRA // N_CHUNK
KT = D // 128

ALU = mybir.AluOpType
ACTF = mybir.ActivationFunctionType

_BUILT = None


def _split_waits(nc):
    """Walrus in this container encodes at most one sync-wait per compute
    instruction. Split extras onto same-engine Drain carriers placed just
    before (engines execute in program order, so semantics are identical)."""
    ctr = [0]

    def process(block):
        new = []
        for inst in block.instructions:
            si = inst.sync_info
            waits = list(si.on_wait) if si is not None and si.on_wait else []
            if len(waits) > 1:
                for w in waits[:-1]:
                    ctr[0] += 1
                    nop = mybir.InstDrain(
                        name=f"waitnop-{ctr[0]}",
                        engine=inst.engine,
                        ins=[],
                        outs=[],
                        sync_info=mybir.SyncInfo(on_wait=[w], on_update=[]),
                    )
                    new.append(nop)
                inst.sync_info = mybir.SyncInfo(
                    on_wait=[waits[-1]], on_update=list(si.on_update or [])
                )
            new.append(inst)
        block.instructions = new
        for b in getattr(block, "blocks", []) or []:
            process(b)

    for b in nc.m.functions[0].blocks:
        process(b)


def _build_module():
    nc = bass.Bass()

    refT8 = nc.declare_dram_parameter("refT8", [128, KT, 128], FP8,
                                      isOutput=False)
    extT8 = nc.declare_dram_parameter("extT8", [128, KT, NEXTRA], FP8,
                                      isOutput=False)
    extb = nc.declare_dram_parameter("extb", [NEXTRA, D], BF16,
                                     isOutput=False)
    # sm rows: 0..3 = x_f (rt, st0, st1, st2); 4..7 = y_f (rs, ss0..2)
    sm_d = nc.declare_dram_parameter("small", [128, 8, D], BF16,
                                     isOutput=False)
    hub_d = nc.declare_dram_parameter("hub", [128, N_UNITS], F32,
                                      isOutput=True)

    with tile.TileContext(nc) as tc:
        with (
            tc.tile_pool(name="singles", bufs=1) as singles,
            tc.tile_pool(name="ext", bufs=8) as ext,
            tc.tile_pool(name="ps", bufs=8, space="PSUM") as ps,
            tc.tile_pool(name="ab", bufs=2) as ab,
        ):
            dma = nc.sync.dma_start

            refT_sb = singles.tile([128, KT, 128], FP8)
            dma(out=refT_sb, in_=refT8.ap())
            sm = singles.tile([128, 8, D], BF16)

            sh = singles.tile([128, TOPK, D], BF16)
            cv = singles.tile([128, 4, D], BF16)       # c2, c3_0..2
            cvd1 = singles.tile([128, NFRAMES, D], BF16)
            d1a = singles.tile([128, NFRAMES, D], BF16)
            e_y = singles.tile([128, 4, D], BF16)      # exp(+y_f)
            d1b = singles.tile([128, NFRAMES, D], BF16)
            e_nsh = singles.tile([128, TOPK, D], BF16)     # exp(-sh_j)
            cand = singles.tile([128, N_CHUNK * 8], F32)
            enc = singles.tile([128, N_CHUNK * 8], F32)
            top8 = singles.tile([128, 8], F32)
            idx32 = singles.tile([128, TOPK], I32)

            SaAll = singles.tile([128, N_UNITS], F32)
            SbAll = singles.tile([128, N_UNITS], F32)
            SAll = singles.tile([128, N_UNITS], F32)

            def dve_sum(in0, in1, acc, tag, name):
                """acc = sum(in0*in1): DVE TT product + 4x tensor_scalar
                accum pass."""
                prod = ab.tile([128, D], BF16, tag=tag, name=f"p{name}")
                nc.vector.tensor_mul(prod, in0, in1)
                junk = ab.tile([128, D], BF16, tag=tag + "j",
                               name=f"j{name}")
                nc.vector.tensor_scalar(junk, prod, 1.0, 0.0, op0=ALU.mult,
                                        op1=ALU.add, accum_out=acc)

            def pool_sum(in0, in1, acc, name):
                """acc = sum(in0*in1): Pool TT product + DVE 4x tsacc pass
                (walrus only accepts TensorTensor-class ops on Pool)."""
                prod = ab.tile([128, D], BF16, tag="pb", name=f"q{name}")
                nc.gpsimd.tensor_mul(prod, in0, in1)
                junk = ab.tile([128, D], BF16, tag="pbj", name=f"qj{name}")
                nc.vector.tensor_scalar(junk, prod, 1.0, 0.0, op0=ALU.mult,
                                        op1=ALU.add, accum_out=acc)

            pts = [
                ps.tile([128, CHUNK], F32, tag="pt", name=f"pt{c}")
                for c in range(N_CHUNK)
            ]

            def chunk_block(c):
                et = ext.tile([128, KT, CHUNK], FP8, tag="et", name=f"et{c}")
                dma(out=et, in_=extT8.ap()[:, :, c * CHUNK:(c + 1) * CHUNK])
                pt = pts[c]
                # DoubleRow folds k-tile pairs: out = sum_i W[:,i].T @ X[:,i]
                for kp in range(KT // 2):
                    nc.tensor.matmul(
                        pt,
                        lhsT=refT_sb[:, 2 * kp:2 * kp + 2, :],
                        rhs=et[:, 2 * kp:2 * kp + 2, :],
                        start=(kp == 0), stop=(kp == KT // 2 - 1),
                        perf_mode=mybir.MatmulPerfMode.DoubleRow,
                    )
                c8 = slice(c * 8, (c + 1) * 8)
                with tc.high_priority():
                    nc.vector.max(cand[:, c8], pt)
                    ci = ab.tile([128, 8], U16, tag="ci", name=f"ci{c}")
                    nc.vector.max_index(ci, cand[:, c8], pt)
                # enc = (round(4*v)+1536)*4096 + (ci + CHUNK*c); all integer-
                # valued fp32 < 2^24 so exact; unique since indices differ.
                # The tiny fold ops ride the Pool engine.
                gi = ab.tile([128, 8], F32, tag="gi", name=f"gi{c}")
                nc.vector.tensor_scalar(gi, ci, 1.0, float(CHUNK * c),
                                        op0=ALU.mult, op1=ALU.add)
                q = ab.tile([128, 8], I32, tag="q", name=f"q{c}")
                nc.vector.tensor_scalar(q, cand[:, c8], 4.0, 1536.0,
                                        op0=ALU.mult, op1=ALU.add)
                qf = ab.tile([128, 8], F32, tag="qf", name=f"qf{c}")
                nc.vector.tensor_scalar(qf, qf_in := q, 4096.0, None,
                                        op0=ALU.mult)
                nc.vector.tensor_add(enc[:, c8], qf, gi)

            for c in range(2):
                chunk_block(c)
            # small rides between chunks 1 and 2: the first enc blocks start
            # ~6us earlier while prep work still lands mid-stream
            dma(out=sm, in_=sm_d.ap())
            for c in range(2, N_CHUNK):
                chunk_block(c)

            # ---- mid-stream prep (lands as sm arrives): cv on Pool, d1
            # sides on DVE/ACT; all pre-gather
            nc.gpsimd.tensor_sub(cv, sm[:, 0:4, :], sm[:, 4:8, :])
            nc.vector.tensor_sub(d1a, sm[:, 0:1, :].broadcast_to(
                [128, NFRAMES, D]), sm[:, 1:4, :])
            nc.vector.tensor_sub(d1b, sm[:, 4:5, :].broadcast_to(
                [128, NFRAMES, D]), sm[:, 5:8, :])
            nc.scalar.activation(e_y, sm[:, 4:8, :], ACTF.Exp)
            ead1 = singles.tile([128, NFRAMES, D], BF16)
            ebd1 = singles.tile([128, NFRAMES, D], BF16)
            for f in range(NFRAMES):
                nc.scalar.activation(ead1[:, f, :], d1a[:, f, :], ACTF.Exp,
                                     accum_out=SaAll[:, f:f + 1])
                nc.scalar.activation(ebd1[:, f, :], d1b[:, f, :], ACTF.Exp,
                                     accum_out=SbAll[:, f:f + 1])

            # ---- merge + arithmetic index decode (decode on Pool: it sits
            # directly before the gather launches on the same queue) --------
            with tc.high_priority():
                nc.vector.max(top8, enc)
                yi = singles.tile([128, TOPK], I32)
                nc.vector.tensor_scalar(yi, top8[:, :TOPK], 1.0 / 4096.0,
                                        -0.499, op0=ALU.mult, op1=ALU.add)
                yf = singles.tile([128, TOPK], F32)
                nc.vector.tensor_scalar(yf, yi, 4096.0, None, op0=ALU.mult)
                idxf = singles.tile([128, TOPK], F32)
                nc.vector.tensor_sub(idxf, top8[:, :TOPK], yf)
                nc.vector.tensor_copy(idx32, idxf)

            with tc.high_priority():
                for j in range(TOPK):
                    nc.gpsimd.indirect_dma_start(
                        out=sh[:, j, :],
                        out_offset=None,
                        in_=extb.ap(),
                        in_offset=bass.IndirectOffsetOnAxis(
                            ap=idx32[:, j:j + 1], axis=0
                        ),
                    )

            # ---- d2/d3 units: (j, f) ------------------------------------
            # Sa: direct (DVE sub + ACT exp w/ accum_out, also yields ea)
            # S : sum(ea * cv_f)       (DVE TT + tsacc)
            # Sb: sum(e_y_f * e_nsh_j) (Pool fused stt; 2 units direct)
            for j in range(TOPK):
                nc.scalar.activation(e_nsh[:, j, :], sh[:, j, :], ACTF.Exp,
                                     scale=-1.0)
                av = [ab.tile([128, D], BF16, tag=f"a{f}", name=f"av{j}_{f}")
                      for f in range(4)]
                for f in range(4):
                    nc.vector.tensor_sub(av[f], sm[:, f, :], sh[:, j, :])
                eav = []
                for f in range(4):
                    col = 3 + 4 * j + f
                    ea = ab.tile([128, D], BF16, tag=f"ea{f}",
                                 name=f"eav{j}_{f}")
                    nc.scalar.activation(ea, av[f], ACTF.Exp,
                                         accum_out=SaAll[:, col:col + 1])
                    eav.append(ea)
                for f in range(4):
                    col = 3 + 4 * j + f
                    dve_sum(eav[f], cv[:, f, :], SAll[:, col:col + 1],
                            "ps", f"s{j}{f}")
                    if j == 3 and f >= 2:
                        # Sb direct: sub + exp with accum (ACT has slack)
                        bv = ab.tile([128, D], BF16, tag="bv",
                                     name=f"bv{j}_{f}")
                        nc.vector.tensor_sub(bv, sm[:, 4 + f, :],
                                             sh[:, j, :])
                        eb = ab.tile([128, D], BF16, tag="eb",
                                     name=f"ebv{j}_{f}")
                        nc.scalar.activation(eb, bv, ACTF.Exp,
                                             accum_out=SbAll[:, col:col + 1])
                    else:
                        pool_sum(e_y[:, f, :], e_nsh[:, j, :],
                                 SbAll[:, col:col + 1], f"b{j}{f}")

                if j == 2:
                    # cvd1 + d1 S-sums slot into unit-phase DVE bubbles
                    nc.vector.tensor_sub(cvd1, cv[:, 0:1, :].broadcast_to(
                        [128, NFRAMES, D]), cv[:, 1:4, :])
                    for f in range(NFRAMES):
                        dve_sum(ead1[:, f, :], cvd1[:, f, :],
                                SAll[:, f:f + 1], "s1", f"s1{f}")

            # ---- tail: kl, smooth-l1, writeback -------------------------
            recip = singles.tile([128, N_UNITS], F32)
            nc.vector.reciprocal(recip, SaAll)
            kl = singles.tile([128, N_UNITS], F32)
            nc.vector.tensor_mul(kl, SAll, recip)
            lnsa = singles.tile([128, N_UNITS], F32)
            nc.scalar.activation(lnsa, SaAll, ACTF.Ln)
            lnsb = singles.tile([128, N_UNITS], F32)
            nc.scalar.activation(lnsb, SbAll, ACTF.Ln)
            nc.vector.tensor_sub(kl, kl, lnsa)
            nc.vector.tensor_add(kl, kl, lnsb)

            kl2 = singles.tile([128, N_UNITS], F32)
            nc.vector.tensor_mul(kl2, kl, kl)
            km = singles.tile([128, N_UNITS], F32)
            nc.vector.tensor_scalar(km, kl, 0.25, None, op0=ALU.subtract)
            mask = singles.tile([128, N_UNITS], mybir.dt.uint8)
            nc.vector.tensor_scalar(mask, kl, 0.5, None, op0=ALU.is_lt)
            hub = singles.tile([128, N_UNITS], F32)
            nc.vector.select(hub, mask, kl2, km)
            dma(out=hub_d.ap(), in_=hub)

    _split_waits(nc)
    return nc


def get_module():
    global _BUILT
    if _BUILT is None:
        _BUILT = _build_module()
    return _BUILT


def make_in_maps(teacher_feats, student_feats, ref_perm, shared_perm):
    """Host-side sharding: slice/normalize/layout the per-core inputs."""
    import ml_dtypes

    tf = np.ascontiguousarray(np.asarray(teacher_feats, dtype=np.float32))
    sf = np.ascontiguousarray(np.asarray(student_feats, dtype=np.float32))
    rp = np.asarray(ref_perm, dtype=np.int64)
    sp = np.asarray(shared_perm, dtype=np.int64)[:NUM_REF]

    in_maps = []
    for b in range(B):
        extra = np.ascontiguousarray(
            tf[b, list(EXTRA_FRAMES)].reshape(NEXTRA, D)
        )
        en = np.maximum(
            np.sqrt((extra ** 2).sum(axis=1)), 1e-12
        ).astype(np.float32)
        extn16 = (extra / en[:, None]) * 16.0
        # [D, NEXTRA] -> [128, KT, NEXTRA] (partition-major k-tiles)
        extT8 = np.ascontiguousarray(
            extn16.T.reshape(KT, 128, NEXTRA).transpose(1, 0, 2)
        ).astype(ml_dtypes.float8_e4m3)
        extb = extra.astype(ml_dtypes.bfloat16)

        ref_t = tf[b, 0][rp]
        ref_s = sf[b, 0][rp]
        rn = np.maximum(
            np.sqrt((ref_t ** 2).sum(axis=1, keepdims=True)), 1e-12
        ).astype(np.float32)
        refn16 = (ref_t / rn) * 16.0
        st_all = np.stack([tf[b, t][sp] for t in SHARED_T])   # [3, 256, D]
        ss_all = np.stack([sf[b, s][sp] for s in SHARED_S])
        for h in range(2):
            sl = slice(h * NREF_CORE, (h + 1) * NREF_CORE)
            refT8 = np.ascontiguousarray(
                refn16[sl].T.reshape(KT, 128, 128).transpose(1, 0, 2)
            ).astype(ml_dtypes.float8_e4m3)
            # x rows then y rows: [rt, st0, st1, st2, rs, ss0, ss1, ss2]
            small = np.stack(
                [ref_t[sl], st_all[0][sl], st_all[1][sl], st_all[2][sl],
                 ref_s[sl], ss_all[0][sl], ss_all[1][sl], ss_all[2][sl]],
                axis=1,
            ).astype(ml_dtypes.bfloat16)
            in_maps.append(
                dict(
                    refT8=refT8,
                    extT8=extT8,
                    extb=extb,
                    small=np.ascontiguousarray(small),
                )
            )
    return in_maps


def finish(hub_stack):
    """hub_stack: [8, 128, 19] per-core smooth-l1 values -> scalar loss."""
    hs = np.asarray(hub_stack, dtype=np.float64)
    d1 = hs[..., 0:3].sum()
    d2 = hs[..., [3, 7, 11, 15]].sum()
    d3 = hs[..., [4, 5, 6, 8, 9, 10, 12, 13, 14, 16, 17, 18]].sum()
    n_d1 = NFRAMES * B * NUM_REF                 # 3072
    n_d2 = B * NUM_REF * TOPK                    # 4096 (dedup: loop adds 3x)
    n_d3 = NFRAMES * B * NUM_REF * TOPK          # 12288
    return np.float32(d1 / n_d1 + d2 / n_d2 + d3 / n_d3)


def run(in_maps, trace=False):
    nc = get_module()
    res = run_bass_kernel_spmd(nc, in_maps, list(range(8)), trace=trace)
    return res


def kernel(teacher_feats, student_feats, ref_perm, shared_perm):
    in_maps = make_in_maps(teacher_feats, student_feats, ref_perm, shared_perm)
    res = run(in_maps)
    hub = np.stack([r["hub"] for r in res.results])
    return finish(hub)


# revision 34
# speedup vs baseline: 1.0141x; 1.0141x over previous
"""Trainium2 Bass kernel for nn_DA3CrossFrameRKDDistanceLoss (v3).

Math (reference semantics): ref rows (teacher/student frame 0, ref_perm
subsample), extra = teacher frames [1,3,5,7] concat -> [4096, D].  Cosine
top-4 neighbours of each ref row inside extra; KL(softmax(diff_t) ||
softmax(diff_s)) per row with diff pairs (d1: ref-shared, d2: ref-simhigh,
d3: shared-simhigh), smooth-L1 (beta=0.5) of each KL, averaged per branch
and summed.  kl = S/Sa - ln Sa + ln Sb with a = diff_t, b = diff_s,
S = sum(exp(a)*(a-b)), Sa = sum(exp(a)), Sb = sum(exp(b)).  d2 in the
reference loop is identical across the 3 shared frames -> computed once.

Sharding: 8 cores = (batch b in 0..3) x (half h of the 256 ref rows); no
collectives — the host sums the per-core smooth-L1 outputs.

Device pipeline per core:
  1. fp8(e4m3) similarity matmul (refn*16 @ extn*16.T), streamed in 8
     PSUM-bank chunks; per chunk: top8 values (max8) + in-chunk indices
     (max_index) directly from PSUM, folded into an order-preserving
     encoding enc = (round(4*v)+1536)*4096 + global_idx (fp32-exact
     integers; ties impossible since indices differ).  Numerically
     validated: fp8+quantized selection moves the loss by ~1e-4 rel.
  2. Global top4 = first 4 of max8(enc); index decoded arithmetically
     (no 4096-wide max_index on the critical path).
  3. Four indirect row gathers of the winners from a bf16 copy of extra.
  4. KL phase: a-sides direct (sub -> ACT exp with accum_out giving Sa
     for free); b-sides shared-exponential (exp(y - sh) = e_y * e_nsh,
     product on DVE/Pool, sum via tensor_scalar accum_out at 4x);
     S = sum(ea * cv) likewise.  e_y / e_nss / cv / c2-c3 are computed
     once, batched, during the gather window together with the d1 units.
  5. smooth-L1 tail -> hub [128, 19] -> host reduction.

Container quirks honoured here: walrus allows ONE sync-wait per compute
instruction (_split_waits rewrites extras onto same-engine Drain carriers);
tensor_tensor_reduce / custom-DVE ops hit 'ISA wrong length'; ALU mod is
rejected; max8/max_index only safe on fp32; multi-offset indirect DMA
fails — but tensor_scalar/scalar_tensor_tensor with accum_out DO work.
"""

import os
import sys

import numpy as np

for _p in ("/opt/trn_rl_repo", "/root/.axon_site/_ro/trn_rl_repo"):
    # later inserts go to the front: prefer the axon-site copy when present
    if os.path.isdir(_p) and _p not in sys.path:
        sys.path.insert(0, _p)

import concourse.bass as bass
import concourse.tile as tile
from concourse import mybir
from concourse.bass_utils import run_bass_kernel_spmd

F32 = mybir.dt.float32
BF16 = mybir.dt.bfloat16
FP8 = mybir.dt.float8e4
I32 = mybir.dt.int32
U16 = mybir.dt.uint16

B = 4
P = 1024
D = 1024
NUM_REF = 256
TOPK = 4
NREF_CORE = 128
NEXTRA = 4 * P
EXTRA_FRAMES = (1, 3, 5, 7)
SHARED_T = (2, 4, 6)
SHARED_S = (1, 2, 3)
NFRAMES = 3
N_UNITS = 19              # 3 d1 + 16 (j,f): col = 3 + 4j + f
N_CHUNK = 8
CHUNK = NEXTRA // N_CHUNK
KT = D // 128

ALU = mybir.AluOpType
ACTF = mybir.ActivationFunctionType

_BUILT = None


def _split_waits(nc):
    """Walrus in this container encodes at most one sync-wait per compute
    instruction. Split extras onto same-engine Drain carriers placed just
    before (engines execute in program order, so semantics are identical)."""
    ctr = [0]

    def process(block):
        new = []
        for inst in block.instructions:
            si = inst.sync_info
            waits = list(si.on_wait) if si is not None and si.on_wait else []
            if len(waits) > 1:
                for w in waits[:-1]:
                    ctr[0] += 1
                    nop = mybir.InstDrain(
                        name=f"waitnop-{ctr[0]}",
                        engine=inst.engine,
                        ins=[],
                        outs=[],
                        sync_info=mybir.SyncInfo(on_wait=[w], on_update=[]),
                    )
                    new.append(nop)
                inst.sync_info = mybir.SyncInfo(
                    on_wait=[waits[-1]], on_update=list(si.on_update or [])
                )
            new.append(inst)
        block.instructions = new
        for b in getattr(block, "blocks", []) or []:
            process(b)

    for b in nc.m.functions[0].blocks:
        process(b)


def _build_module():
    nc = bass.Bass()

    refT8 = nc.declare_dram_parameter("refT8", [128, KT, 128], FP8,
                                      isOutput=False)
    extT8 = nc.declare_dram_parameter("extT8", [128, KT, NEXTRA], FP8,
                                      isOutput=False)
    extb = nc.declare_dram_parameter("extb", [NEXTRA, D], BF16,
                                     isOutput=False)
    # sm rows: 0..3 = x_f (rt, st0, st1, st2); 4..7 = y_f (rs, ss0..2)
    sm_d = nc.declare_dram_parameter("small", [128, 8, D], BF16,
                                     isOutput=False)
    hub_d = nc.declare_dram_parameter("hub", [128, N_UNITS], F32,
                                      isOutput=True)

    with tile.TileContext(nc) as tc:
        with (
            tc.tile_pool(name="singles", bufs=1) as singles,
            tc.tile_pool(name="ext", bufs=8) as ext,
            tc.tile_pool(name="ps", bufs=1, space="PSUM") as ps,
            tc.tile_pool(name="ab", bufs=2) as ab,
        ):
            dma = nc.sync.dma_start

            refT_sb = singles.tile([128, KT, 128], FP8)
            dma(out=refT_sb, in_=refT8.ap())
            sm = singles.tile([128, 8, D], BF16)

            sh = singles.tile([128, TOPK, D], BF16)
            cv = singles.tile([128, 4, D], BF16)       # c2, c3_0..2
            cvd1 = singles.tile([128, NFRAMES, D], BF16)
            d1a = singles.tile([128, NFRAMES, D], BF16)
            e_y = singles.tile([128, 4, D], BF16)      # exp(+y_f)
            d1b = singles.tile([128, NFRAMES, D], BF16)
            e_nsh = singles.tile([128, TOPK, D], BF16)     # exp(-sh_j)
            cand = singles.tile([128, N_CHUNK * 8], F32)
            enc = singles.tile([128, N_CHUNK * 8], F32)
            top8 = singles.tile([128, 8], F32)
            idx32 = singles.tile([128, TOPK], I32)

            SaAll = singles.tile([128, N_UNITS], F32)
            SbAll = singles.tile([128, N_UNITS], F32)
            SAll = singles.tile([128, N_UNITS], F32)

            def dve_sum(in0, in1, acc, tag, name):
                """acc = sum(in0*in1): DVE TT product + 4x tensor_scalar
                accum pass."""
                prod = ab.tile([128, D], BF16, tag=tag, name=f"p{name}")
                nc.vector.tensor_mul(prod, in0, in1)
                junk = ab.tile([128, D], BF16, tag=tag + "j",
                               name=f"j{name}")
                nc.vector.tensor_scalar(junk, prod, 1.0, 0.0, op0=ALU.mult,
                                        op1=ALU.add, accum_out=acc)

            def pool_sum(in0, in1, acc, name):
                """acc = sum(in0*in1): Pool TT product + DVE 4x tsacc pass
                (walrus only accepts TensorTensor-class ops on Pool)."""
                prod = ab.tile([128, D], BF16, tag="pb", name=f"q{name}")
                nc.gpsimd.tensor_mul(prod, in0, in1)
                junk = ab.tile([128, D], BF16, tag="pbj", name=f"qj{name}")
                nc.vector.tensor_scalar(junk, prod, 1.0, 0.0, op0=ALU.mult,
                                        op1=ALU.add, accum_out=acc)

            pts = [
                ps.tile([128, CHUNK], F32, tag="pt", name=f"pt{c}")
                for c in range(N_CHUNK)
            ]

            def chunk_block(c):
                et = ext.tile([128, KT, CHUNK], FP8, tag="et", name=f"et{c}")
                dma(out=et, in_=extT8.ap()[:, :, c * CHUNK:(c + 1) * CHUNK])
                pt = pts[c]
                # DoubleRow folds k-tile pairs: out = sum_i W[:,i].T @ X[:,i]
                for kp in range(KT // 2):
                    nc.tensor.matmul(
                        pt,
                        lhsT=refT_sb[:, 2 * kp:2 * kp + 2, :],
                        rhs=et[:, 2 * kp:2 * kp + 2, :],
                        start=(kp == 0), stop=(kp == KT // 2 - 1),
                        perf_mode=mybir.MatmulPerfMode.DoubleRow,
                    )
                c8 = slice(c * 8, (c + 1) * 8)
                with tc.high_priority():
                    nc.vector.max(cand[:, c8], pt)
                    ci = ab.tile([128, 8], U16, tag="ci", name=f"ci{c}")
                    nc.vector.max_index(ci, cand[:, c8], pt)
                # enc = (round(4*v)+1536)*4096 + (ci + CHUNK*c); all integer-
                # valued fp32 < 2^24 so exact; unique since indices differ.
                # The tiny fold ops ride the Pool engine.
                gi = ab.tile([128, 8], F32, tag="gi", name=f"gi{c}")
                nc.vector.tensor_scalar(gi, ci, 1.0, float(CHUNK * c),
                                        op0=ALU.mult, op1=ALU.add)
                q = ab.tile([128, 8], I32, tag="q", name=f"q{c}")
                nc.vector.tensor_scalar(q, cand[:, c8], 4.0, 1536.0,
                                        op0=ALU.mult, op1=ALU.add)
                qf = ab.tile([128, 8], F32, tag="qf", name=f"qf{c}")
                nc.vector.tensor_scalar(qf, qf_in := q, 4096.0, None,
                                        op0=ALU.mult)
                nc.vector.tensor_add(enc[:, c8], qf, gi)

            for c in range(2):
                chunk_block(c)
            # small rides between chunks 1 and 2: the first enc blocks start
            # ~6us earlier while prep work still lands mid-stream
            dma(out=sm, in_=sm_d.ap())
            for c in range(2, N_CHUNK):
                chunk_block(c)

            # ---- mid-stream prep (lands as sm arrives): cv on Pool, d1
            # sides on DVE/ACT; all pre-gather
            nc.gpsimd.tensor_sub(cv, sm[:, 0:4, :], sm[:, 4:8, :])
            nc.vector.tensor_sub(d1a, sm[:, 0:1, :].broadcast_to(
                [128, NFRAMES, D]), sm[:, 1:4, :])
            nc.vector.tensor_sub(d1b, sm[:, 4:5, :].broadcast_to(
                [128, NFRAMES, D]), sm[:, 5:8, :])
            nc.scalar.activation(e_y, sm[:, 4:8, :], ACTF.Exp)
            ead1 = singles.tile([128, NFRAMES, D], BF16)
            ebd1 = singles.tile([128, NFRAMES, D], BF16)
            for f in range(NFRAMES):
                nc.scalar.activation(ead1[:, f, :], d1a[:, f, :], ACTF.Exp,
                                     accum_out=SaAll[:, f:f + 1])
                nc.scalar.activation(ebd1[:, f, :], d1b[:, f, :], ACTF.Exp,
                                     accum_out=SbAll[:, f:f + 1])

            # ---- merge + arithmetic index decode (decode on Pool: it sits
            # directly before the gather launches on the same queue) --------
            with tc.high_priority():
                nc.vector.max(top8, enc)
                yi = singles.tile([128, TOPK], I32)
                nc.vector.tensor_scalar(yi, top8[:, :TOPK], 1.0 / 4096.0,
                                        -0.499, op0=ALU.mult, op1=ALU.add)
                yf = singles.tile([128, TOPK], F32)
                nc.vector.tensor_scalar(yf, yi, 4096.0, None, op0=ALU.mult)
                idxf = singles.tile([128, TOPK], F32)
                nc.vector.tensor_sub(idxf, top8[:, :TOPK], yf)
                nc.vector.tensor_copy(idx32, idxf)

            with tc.high_priority():
                for j in range(TOPK):
                    nc.gpsimd.indirect_dma_start(
                        out=sh[:, j, :],
                        out_offset=None,
                        in_=extb.ap(),
                        in_offset=bass.IndirectOffsetOnAxis(
                            ap=idx32[:, j:j + 1], axis=0
                        ),
                    )

            # ---- d2/d3 units: (j, f) ------------------------------------
            # Sa: direct (DVE sub + ACT exp w/ accum_out, also yields ea)
            # S : sum(ea * cv_f)       (DVE TT + tsacc)
            # Sb: sum(e_y_f * e_nsh_j) (Pool fused stt; 2 units direct)
            for j in range(TOPK):
                nc.scalar.activation(e_nsh[:, j, :], sh[:, j, :], ACTF.Exp,
                                     scale=-1.0)
                av = [ab.tile([128, D], BF16, tag=f"a{f}", name=f"av{j}_{f}")
                      for f in range(4)]
                for f in range(4):
                    nc.vector.tensor_sub(av[f], sm[:, f, :], sh[:, j, :])
                eav = []
                for f in range(4):
                    col = 3 + 4 * j + f
                    ea = ab.tile([128, D], BF16, tag=f"ea{f}",
                                 name=f"eav{j}_{f}")
                    nc.scalar.activation(ea, av[f], ACTF.Exp,
                                         accum_out=SaAll[:, col:col + 1])
                    eav.append(ea)
                for f in range(4):
                    col = 3 + 4 * j + f
                    dve_sum(eav[f], cv[:, f, :], SAll[:, col:col + 1],
                            "ps", f"s{j}{f}")
                    if j == 3 and f >= 2:
                        # Sb direct: sub + exp with accum (ACT has slack)
                        bv = ab.tile([128, D], BF16, tag="bv",
                                     name=f"bv{j}_{f}")
                        nc.vector.tensor_sub(bv, sm[:, 4 + f, :],
                                             sh[:, j, :])
                        eb = ab.tile([128, D], BF16, tag="eb",
                                     name=f"ebv{j}_{f}")
                        nc.scalar.activation(eb, bv, ACTF.Exp,
                                             accum_out=SbAll[:, col:col + 1])
                    else:
                        pool_sum(e_y[:, f, :], e_nsh[:, j, :],
                                 SbAll[:, col:col + 1], f"b{j}{f}")

                if j == 2:
                    # cvd1 + d1 S-sums slot into unit-phase DVE bubbles
                    nc.vector.tensor_sub(cvd1, cv[:, 0:1, :].broadcast_to(
                        [128, NFRAMES, D]), cv[:, 1:4, :])
                    for f in range(NFRAMES):
                        dve_sum(ead1[:, f, :], cvd1[:, f, :],
                                SAll[:, f:f + 1], "s1", f"s1{f}")

            # ---- tail: kl, smooth-l1, writeback -------------------------
            recip = singles.tile([128, N_UNITS], F32)
            nc.vector.reciprocal(recip, SaAll)
            kl = singles.tile([128, N_UNITS], F32)
            nc.vector.tensor_mul(kl, SAll, recip)
            lnsa = singles.tile([128, N_UNITS], F32)
            nc.scalar.activation(lnsa, SaAll, ACTF.Ln)
            lnsb = singles.tile([128, N_UNITS], F32)
            nc.scalar.activation(lnsb, SbAll, ACTF.Ln)
            nc.vector.tensor_sub(kl, kl, lnsa)
            nc.vector.tensor_add(kl, kl, lnsb)

            kl2 = singles.tile([128, N_UNITS], F32)
            nc.vector.tensor_mul(kl2, kl, kl)
            km = singles.tile([128, N_UNITS], F32)
            nc.vector.tensor_scalar(km, kl, 0.25, None, op0=ALU.subtract)
            mask = singles.tile([128, N_UNITS], mybir.dt.uint8)
            nc.vector.tensor_scalar(mask, kl, 0.5, None, op0=ALU.is_lt)
            hub = singles.tile([128, N_UNITS], F32)
            nc.vector.select(hub, mask, kl2, km)
            dma(out=hub_d.ap(), in_=hub)

    _split_waits(nc)
    return nc


def get_module():
    global _BUILT
    if _BUILT is None:
        _BUILT = _build_module()
    return _BUILT


def make_in_maps(teacher_feats, student_feats, ref_perm, shared_perm):
    """Host-side sharding: slice/normalize/layout the per-core inputs."""
    import ml_dtypes

    tf = np.ascontiguousarray(np.asarray(teacher_feats, dtype=np.float32))
    sf = np.ascontiguousarray(np.asarray(student_feats, dtype=np.float32))
    rp = np.asarray(ref_perm, dtype=np.int64)
    sp = np.asarray(shared_perm, dtype=np.int64)[:NUM_REF]

    in_maps = []
    for b in range(B):
        extra = np.ascontiguousarray(
            tf[b, list(EXTRA_FRAMES)].reshape(NEXTRA, D)
        )
        en = np.maximum(
            np.sqrt((extra ** 2).sum(axis=1)), 1e-12
        ).astype(np.float32)
        extn16 = (extra / en[:, None]) * 16.0
        # [D, NEXTRA] -> [128, KT, NEXTRA] (partition-major k-tiles)
        extT8 = np.ascontiguousarray(
            extn16.T.reshape(KT, 128, NEXTRA).transpose(1, 0, 2)
        ).astype(ml_dtypes.float8_e4m3)
        extb = extra.astype(ml_dtypes.bfloat16)

        ref_t = tf[b, 0][rp]
        ref_s = sf[b, 0][rp]
        rn = np.maximum(
            np.sqrt((ref_t ** 2).sum(axis=1, keepdims=True)), 1e-12
        ).astype(np.float32)
        refn16 = (ref_t / rn) * 16.0
        st_all = np.stack([tf[b, t][sp] for t in SHARED_T])   # [3, 256, D]
        ss_all = np.stack([sf[b, s][sp] for s in SHARED_S])
        for h in range(2):
            sl = slice(h * NREF_CORE, (h + 1) * NREF_CORE)
            refT8 = np.ascontiguousarray(
                refn16[sl].T.reshape(KT, 128, 128).transpose(1, 0, 2)
            ).astype(ml_dtypes.float8_e4m3)
            # x rows then y rows: [rt, st0, st1, st2, rs, ss0, ss1, ss2]
            small = np.stack(
                [ref_t[sl], st_all[0][sl], st_all[1][sl], st_all[2][sl],
                 ref_s[sl], ss_all[0][sl], ss_all[1][sl], ss_all[2][sl]],
                axis=1,
            ).astype(ml_dtypes.bfloat16)
            in_maps.append(
                dict(
                    refT8=refT8,
                    extT8=extT8,
                    extb=extb,
                    small=np.ascontiguousarray(small),
                )
            )
    return in_maps


def finish(hub_stack):
    """hub_stack: [8, 128, 19] per-core smooth-l1 values -> scalar loss."""
    hs = np.asarray(hub_stack, dtype=np.float64)
    d1 = hs[..., 0:3].sum()
    d2 = hs[..., [3, 7, 11, 15]].sum()
    d3 = hs[..., [4, 5, 6, 8, 9, 10, 12, 13, 14, 16, 17, 18]].sum()
    n_d1 = NFRAMES * B * NUM_REF                 # 3072
    n_d2 = B * NUM_REF * TOPK                    # 4096 (dedup: loop adds 3x)
    n_d3 = NFRAMES * B * NUM_REF * TOPK          # 12288
    return np.float32(d1 / n_d1 + d2 / n_d2 + d3 / n_d3)


def run(in_maps, trace=False):
    nc = get_module()
    res = run_bass_kernel_spmd(nc, in_maps, list(range(8)), trace=trace)
    return res


def kernel(teacher_feats, student_feats, ref_perm, shared_perm):
    in_maps = make_in_maps(teacher_feats, student_feats, ref_perm, shared_perm)
    res = run(in_maps)
    hub = np.stack([r["hub"] for r in res.results])
    return finish(hub)
